# revision 7
# baseline (speedup 1.0000x reference)
"""Trainium2 Bass kernel for a GPT2-style decoder block (causal self-attn +
cross-attn + MLP, post-LN residuals).

Sharding: 8 cores = 4 pairs. Pair p handles batch element p (data parallel over
B=4); within a pair the 1024 tokens are split 512/512 by interleaved 128-blocks
([0,3,4,7] / [1,2,5,6]) so causal-attention work is balanced. K/V for both
attentions are exchanged inside each pair with an AllGather. Activations are
kept feature-major ("T layout": features on partitions, tokens on free) so every
matmul contraction runs over the partition axis; LayerNorm statistics are
computed with ones-vector matmuls on the PE, and the softmax denominator comes
free from a ones-column appended to V.

The SPMD program is identical on all cores; the parity-dependent causal
structure enters only through data (the host-computed `cmask` tensor and the
sharded inputs).
"""
import sys

sys.path.insert(0, '/opt/trn_rl_repo')

import contextlib

import numpy as np
import concourse.bacc as bacc
import concourse.mybir as mybir
import concourse.tile as tile
from concourse.bass_utils import run_bass_kernel_spmd

# ---------------------------------------------------------------- constants
B, S, D, H, HD, FF = 4, 1024, 1024, 16, 64, 4096
SH = 512                 # tokens per core
KD = D // 128            # 8 feature tiles of the model dim
KF = FF // 128           # 32 feature tiles of the MLP hidden dim
NKT = S // 128           # 8 key tiles (full sequence)
EV = [0, 3, 4, 7]        # query blocks of even cores (member 0 of each pair)
OD = [1, 2, 5, 6]        # query blocks of odd cores
KB = EV + OD             # key-tile order in the gathered K/V buffers
J0U = [0, 1, 2, 3, 0, 1, 2, 3]   # first computed q-block per key tile (union)
LN_EPS = 1e-5
MASK_NEG = -1e5
N_CORES = 8

f32 = mybir.dt.float32
f32r = mybir.dt.float32r
AF = mybir.ActivationFunctionType
ALU = mybir.AluOpType

# kv exchange buffer: k section [8, 128, 512] (feature-tile major), then
# v section [4, 128, 1024] (token-block major)
KV_K_ELEMS = NKT * 128 * SH
KV_V_ELEMS = 4 * 128 * D
KV_ELEMS = KV_K_ELEMS + KV_V_ELEMS

WEIGHT_SPECS = [
    ("c_attn_w", [D, 3 * D]), ("c_attn_b", [3 * D]),
    ("self_proj_w", [D, D]), ("self_proj_b", [D]),
    ("q_w", [D, D]), ("q_b", [D]),
    ("kv_w", [D, 2 * D]), ("kv_b", [2 * D]),
    ("cross_proj_w", [D, D]), ("cross_proj_b", [D]),
    ("fc_w", [D, FF]), ("fc_b", [FF]),
    ("mlp_proj_w", [FF, D]), ("mlp_proj_b", [D]),
    ("ln1_g", [D]), ("ln1_b", [D]), ("ln2_g", [D]), ("ln2_b", [D]),
    ("ln3_g", [D]), ("ln3_b", [D]),
]


def build_program():
    nc = bacc.Bacc("TRN2", target_bir_lowering=False, debug=False,
                   num_devices=N_CORES)

    xt_d = nc.dram_tensor("xt", [D, SH], f32, kind="ExternalInput").ap()
    ctxt_d = nc.dram_tensor("ctxt", [D, SH], f32, kind="ExternalInput").ap()
    cmask_d = nc.dram_tensor("cmask", [NKT, 128, 128], f32,
                             kind="ExternalInput").ap()
    w = {}
    for name, shape in WEIGHT_SPECS:
        w[name] = nc.dram_tensor(name, shape, f32, kind="ExternalInput").ap()
    out_d = nc.dram_tensor("out", [D, SH], f32, kind="ExternalOutput").ap()

    with tile.TileContext(nc) as tc:
        build_body(nc, tc, xt_d, ctxt_d, cmask_d, w, out_d)
    nc.compile()
    return nc


def build_body(nc, tc, xt_d, ctxt_d, cmask_d, w, out_d):
    ctx = contextlib.ExitStack()
    const = ctx.enter_context(tc.tile_pool(name="const", bufs=1))
    wp = ctx.enter_context(tc.tile_pool(name="wp", bufs=3))
    resid = ctx.enter_context(tc.tile_pool(name="resid", bufs=2))
    scr = ctx.enter_context(tc.tile_pool(name="scr", bufs=3))
    scr2 = ctx.enter_context(tc.tile_pool(name="scr2", bufs=2))
    small = ctx.enter_context(tc.tile_pool(name="small", bufs=2))
    mm = ctx.enter_context(tc.tile_pool(name="mm", bufs=6, space="PSUM"))
    aux = ctx.enter_context(tc.tile_pool(name="aux", bufs=2, space="PSUM"))
    dram = ctx.enter_context(tc.tile_pool(name="dram", bufs=1, space="DRAM"))

    # ---------------- constants in SBUF
    def load_col_pack(vec_ap, n_tiles, name):
        """[N] vector -> [128, N/128] (column m = bias slice for feature tile m)"""
        t = const.tile([128, n_tiles], f32, tag=name, name=name)
        nc.sync.dma_start(t[:], vec_ap.rearrange("(t p) -> p t", p=128))
        return t

    cab = load_col_pack(w["c_attn_b"][0:2 * D], 16, "cab")       # q,k biases
    spb = load_col_pack(w["self_proj_b"], KD, "spb")
    qb = load_col_pack(w["q_b"], KD, "qb")
    kvbk = load_col_pack(w["kv_b"][0:D], KD, "kvbk")
    cpb = load_col_pack(w["cross_proj_b"], KD, "cpb")
    fcb = load_col_pack(w["fc_b"], KF, "fcb")
    mpb = load_col_pack(w["mlp_proj_b"], KD, "mpb")
    lng = [load_col_pack(w[f"ln{i}_g"], KD, f"ln{i}g") for i in (1, 2, 3)]
    lnb = [load_col_pack(w[f"ln{i}_b"], KD, f"ln{i}b") for i in (1, 2, 3)]

    def load_row_bcast(vec_ap, name):
        """[D] vector -> [128, D] with the vector replicated on every partition"""
        row = const.tile([1, D], f32, tag="vbrow", name=name + "row")
        nc.sync.dma_start(row[:], vec_ap[None, :])
        t = const.tile([128, D], f32, tag="vb", name=name)
        nc.gpsimd.partition_broadcast(t[:], row[:])
        return t

    cmask_sb = const.tile([128, NKT, 128], f32, tag="cmask")
    nc.sync.dma_start(cmask_sb[:], cmask_d.transpose([1, 0, 2]))

    ones_f = const.tile([128, 128], f32, tag="onesf")
    nc.vector.memset(ones_f[:], 1.0)
    eps_sb = const.tile([128, 1], f32, tag="eps")
    nc.vector.memset(eps_sb[:], LN_EPS)
    ones_mm = const.tile([128, 1], f32r, tag="onesmm")
    nc.vector.tensor_copy(ones_mm[:], ones_f[:, 0:1])

    # input activations, feature-major
    xT = resid.tile([128, KD, SH], f32r, tag="resid")
    nc.sync.dma_start(xT[:], xt_d.rearrange("(k p) t -> p k t", p=128).bitcast(f32r))

    # ---------------- generic building blocks
    def wstrip(w_ap, k, c0, cn):
        t = wp.tile([128, cn], f32r, tag="w", name="w")
        nc.sync.dma_start(t[:],
                          w_ap[k * 128:(k + 1) * 128, c0:c0 + cn].bitcast(f32r))
        return t

    def linearT(src, w_ap, m0, mt, kt, consume):
        """feature-major out tiles m in [0,mt): psum_m = sum_k w[:,m0+m].T @ src(k)"""
        for mc in range(0, mt, 4):
            nchunk = min(4, mt - mc)
            psums = [mm.tile([128, SH], f32, tag="mm", name=f"ps{i}")
                     for i in range(nchunk)]
            for k in range(kt):
                ws = wstrip(w_ap, k, (m0 + mc) * 128, nchunk * 128)
                for i in range(nchunk):
                    nc.tensor.matmul(psums[i][:], ws[:, i * 128:(i + 1) * 128],
                                     src(k), start=(k == 0), stop=(k == kt - 1))
            for i in range(nchunk):
                consume(mc + i, psums[i])

    def layer_norm(rT, g_sb, b_sb, outT):
        """outT = LN(rT) * g + b; stats over features via ones-matmuls on PE."""
        ps_sum = aux.tile([1, SH], f32, tag="aux", name="ps_sum")
        ps_sq = aux.tile([1, SH], f32, tag="aux", name="ps_sq")
        for k in range(KD):
            nc.tensor.matmul(ps_sum[:], ones_mm[:], rT[:, k, :],
                             start=(k == 0), stop=(k == KD - 1))
        for k in range(KD):
            sq = scr.tile([128, SH], f32r, tag="sq", name="sq")
            nc.scalar.activation(sq[:], rT[:, k, :], AF.Square)
            nc.tensor.matmul(ps_sq[:], ones_mm[:], sq[:],
                             start=(k == 0), stop=(k == KD - 1))
        mean = small.tile([1, SH], f32, tag="stat", bufs=4, name="mean")
        nc.vector.tensor_scalar_mul(mean[:], ps_sum[:], 1.0 / D)
        var = small.tile([1, SH], f32, tag="stat", bufs=4, name="var")
        # var = ps_sq/D - mean^2, computed as (mean * -mean) + ps_sq/D
        nc.vector.scalar_tensor_tensor(var[:], mean[:], -1.0, mean[:],
                                       op0=ALU.mult, op1=ALU.mult)
        nc.vector.scalar_tensor_tensor(var[:], ps_sq[:], 1.0 / D, var[:],
                                       op0=ALU.mult, op1=ALU.add)
        sd = small.tile([1, SH], f32, tag="stat", bufs=4, name="sd")
        nc.scalar.activation(sd[:], var[:], AF.Sqrt, bias=eps_sb[0:1, :])
        rstd = small.tile([1, SH], f32, tag="stat", bufs=4, name="rstd")
        nc.vector.reciprocal(rstd[:], sd[:])
        mb = scr2.tile([128, SH], f32, tag="mb", name="mb")
        rs = scr2.tile([128, SH], f32, tag="rs", name="rs")
        nc.gpsimd.partition_broadcast(mb[:], mean[:])
        nc.gpsimd.partition_broadcast(rs[:], rstd[:])
        for k in range(KD):
            t = scr.tile([128, SH], f32, tag="lnt", name="lnt")
            nc.vector.tensor_sub(t[:], rT[:, k, :], mb[:])
            nc.vector.tensor_mul(t[:], t[:], rs[:])
            nc.vector.tensor_scalar(outT[:, k, :], t[:],
                                    scalar1=g_sb[:, k:k + 1],
                                    scalar2=b_sb[:, k:k + 1],
                                    op0=ALU.mult, op1=ALU.add)

    def proj_resid_ln(aT_sb, w_ap, bias_sb, xin, g_sb, b_sb, xout):
        rT = resid.tile([128, KD, SH], f32r, tag="resid", name="rT")

        def consume(m, psum):
            nc.vector.scalar_tensor_tensor(rT[:, m, :], psum[:],
                                           bias_sb[:, m:m + 1], xin[:, m, :],
                                           op0=ALU.add, op1=ALU.add)

        linearT(lambda k: aT_sb[:, k, :], w_ap, 0, KD, KD, consume)
        layer_norm(rT, g_sb, b_sb, xout)

    # =================================================== attention phases
    with tc.tile_pool(name="attn", bufs=1) as ap:

        def kv_produce(srcT, w_ap, kcol0, vcol0, kbias, vb_bcast, kv_in):
            """K (feature-major) + V (token-major) for this core's 512 tokens,
            written to the kv_in DRAM buffer for the pair AllGather."""
            kv_flat = kv_in[:]

            def consume_k(m, psum):
                ks = ap.tile([128, SH], f32r, tag="kstage", bufs=3, name="ks")
                nc.vector.tensor_scalar_add(ks[:], psum[:], kbias[:, m:m + 1])
                dst = kv_flat[m * 128 * SH:(m + 1) * 128 * SH]
                nc.sync.dma_start(dst.rearrange("(p t) -> p t", p=128),
                                  ks[:].bitcast(f32))

            linearT(lambda k: srcT[:, k, :], w_ap, kcol0 // 128, KD, KD,
                    consume_k)

            v_dst = kv_flat[KV_K_ELEMS:].rearrange("(b p e) -> b p e",
                                                   p=128, e=D)
            for tb in range(4):
                for vc in range(2):
                    psum = mm.tile([128, SH], f32, tag="mm", name="psv")
                    for k in range(KD):
                        ws = wstrip(w_ap, k, vcol0 + vc * 512, 512)
                        nc.tensor.matmul(psum[:],
                                         srcT[:, k, tb * 128:(tb + 1) * 128],
                                         ws[:], start=(k == 0),
                                         stop=(k == KD - 1))
                    vs = ap.tile([128, SH], f32r, tag="vstage", bufs=3,
                                 name="vs")
                    nc.vector.tensor_add(vs[:], psum[:],
                                         vb_bcast[:, vc * 512:(vc + 1) * 512])
                    nc.sync.dma_start(v_dst[tb, :, vc * 512:(vc + 1) * 512],
                                      vs[:].bitcast(f32))

        def kv_gather(kv_in, tag):
            kv_out = dram.tile([2, KV_ELEMS], f32, tag="kvout" + tag,
                               name="kvout" + tag)
            nc.gpsimd.collective_compute(
                "AllGather", ALU.bypass,
                replica_groups=[[0, 1], [2, 3], [4, 5], [6, 7]],
                ins=[kv_in.opt()],
                outs=[kv_out.opt()],
            )
            return kv_out

        def attention(qT_sb, kv_out, aT_sb, masked):
            for hp in range(H // 2):
                # stream this head-pair's K [128, S] and V [128, 2, NKT, 65]
                kp = ap.tile([128, S], f32r, tag="kp", bufs=2, name="kp")
                vp = ap.tile([128, 2, NKT, 65], f32r, tag="vp", bufs=2,
                             name="vp")
                nc.vector.tensor_copy(
                    vp[:, :, :, 64],
                    ones_f[:, 0:2 * NKT].rearrange("p (a b) -> p a b", a=2))
                for mem in range(2):
                    ksec = kv_out[mem, 0:KV_K_ELEMS].rearrange(
                        "(m p t) -> p m t", p=128, t=SH)
                    nc.sync.dma_start(kp[:, mem * SH:(mem + 1) * SH],
                                      ksec[:, hp, :].bitcast(f32r))
                    vsec = kv_out[mem, KV_K_ELEMS:].rearrange(
                        "(b p h e) -> b p h e", p=128, h=H, e=HD)
                    for tb in range(4):
                        nc.sync.dma_start(
                            vp[:, :, mem * 4 + tb, 0:64],
                            vsec[tb, :, 2 * hp:2 * hp + 2, :].bitcast(f32r))
                for hh in range(2):
                    hr = hh * 64
                    ps_av = aux.tile([65, SH], f32, tag="aux", name="ps_av")
                    for kt in range(NKT):
                        j0 = J0U[kt] if masked else 0
                        q0 = j0 * 128
                        qn = SH - q0
                        ps_s = mm.tile([128, SH], f32, tag="mm", name="ps_s")
                        nc.tensor.matmul(
                            ps_s[:, 0:qn],
                            kp[hr:hr + 64, kt * 128:(kt + 1) * 128],
                            qT_sb[hr:hr + 64, hp, q0:SH],
                            start=True, stop=True)
                        if masked:
                            nc.vector.tensor_add(ps_s[:, 0:128],
                                                 ps_s[:, 0:128],
                                                 cmask_sb[:, kt, :])
                        pr = ap.tile([128, SH], f32r, tag="probs", bufs=4,
                                     name="pr")
                        nc.scalar.activation(pr[:, 0:qn], ps_s[:, 0:qn],
                                             AF.Exp, bias=0.0, scale=0.125)
                        nc.tensor.matmul(
                            ps_av[:, q0:SH],
                            vp[:, hh, kt, :],
                            pr[:, 0:qn],
                            start=(kt == 0), stop=(kt == NKT - 1))
                    # normalize: rows 0:64 = sum(p*v), row 64 = sum(p)
                    rec64 = small.tile([65, SH], f32, tag="rec64",
                                       name="rec64")
                    nc.vector.reciprocal(rec64[64:65, :], ps_av[64:65, :])
                    rec0 = small.tile([1, SH], f32, tag="rec0", name="rec0")
                    nc.sync.dma_start(rec0[:], rec64[64:65, :])
                    rb = small.tile([64, SH], f32, tag="rb", name="rb")
                    nc.gpsimd.partition_broadcast(rb[:], rec0[:])
                    ast = ap.tile([64, SH], f32r, tag="astage", bufs=3,
                                  name="ast")
                    nc.vector.tensor_mul(ast[:], ps_av[0:64, :], rb[:])
                    nc.sync.dma_start(aT_sb[hr:hr + 64, hp, :], ast[:])

        vb_self = load_row_bcast(w["c_attn_b"][2 * D:3 * D], "vbs")

        # ---------- phase 1: causal self-attention
        qT = ap.tile([128, KD, SH], f32r, tag="qT", bufs=1, name="qT")

        def consume_q(m, psum):
            nc.vector.tensor_scalar_add(qT[:, m, :], psum[:], cab[:, m:m + 1])

        linearT(lambda k: xT[:, k, :], w["c_attn_w"], 0, KD, KD, consume_q)

        kv_in_s = dram.tile([KV_ELEMS], f32, tag="kvins", name="kvins")
        kv_produce(xT, w["c_attn_w"], D, 2 * D,
                   kbias=cab[:, 8:16], vb_bcast=vb_self, kv_in=kv_in_s)
        kv_out_s = kv_gather(kv_in_s, "s")

        aT = ap.tile([128, KD, SH], f32r, tag="aT", bufs=1, name="aT")
        attention(qT, kv_out_s, aT, masked=True)

        x1T = resid.tile([128, KD, SH], f32r, tag="resid", name="x1T")
        proj_resid_ln(aT, w["self_proj_w"], spb, xT, lng[0], lnb[0], x1T)

        # ---------- phase 2: cross-attention
        ctxT = ap.tile([128, KD, SH], f32r, tag="ctxT", name="ctxT")
        nc.sync.dma_start(
            ctxT[:], ctxt_d.rearrange("(k p) t -> p k t", p=128).bitcast(f32r))
        vb_cross = load_row_bcast(w["kv_b"][D:2 * D], "vbc")

        q1T = ap.tile([128, KD, SH], f32r, tag="qT", bufs=1, name="q1T")

        def consume_q1(m, psum):
            nc.vector.tensor_scalar_add(q1T[:, m, :], psum[:], qb[:, m:m + 1])

        linearT(lambda k: x1T[:, k, :], w["q_w"], 0, KD, KD, consume_q1)

        kv_in_c = dram.tile([KV_ELEMS], f32, tag="kvinc", name="kvinc")
        kv_produce(ctxT, w["kv_w"], 0, D,
                   kbias=kvbk, vb_bcast=vb_cross, kv_in=kv_in_c)
        kv_out_c = kv_gather(kv_in_c, "c")

        a2T = ap.tile([128, KD, SH], f32r, tag="aT", bufs=1, name="a2T")
        attention(q1T, kv_out_c, a2T, masked=False)

        x2T = resid.tile([128, KD, SH], f32r, tag="resid", name="x2T")
        proj_resid_ln(a2T, w["cross_proj_w"], cpb, x1T, lng[1], lnb[1], x2T)

    # =================================================== phase 3: MLP
    with tc.tile_pool(name="mlp", bufs=1) as mp:
        h_sb = mp.tile([128, KF, SH], f32r, tag="h", name="h_sb")

        def consume_h(m, psum):
            nc.scalar.activation(h_sb[:, m, :], psum[:], AF.Gelu_apprx_tanh,
                                 bias=fcb[:, m:m + 1], scale=1.0)

        linearT(lambda k: x2T[:, k, :], w["fc_w"], 0, KF, KD, consume_h)

        r3T = resid.tile([128, KD, SH], f32r, tag="resid", name="r3T")

        def consume_m(m, psum):
            nc.vector.scalar_tensor_tensor(r3T[:, m, :], psum[:],
                                           mpb[:, m:m + 1], x2T[:, m, :],
                                           op0=ALU.add, op1=ALU.add)

        linearT(lambda k: h_sb[:, k, :], w["mlp_proj_w"], 0, KD, KF, consume_m)

        x3T = resid.tile([128, KD, SH], f32, tag="resid", name="x3T")
        layer_norm(r3T, lng[2], lnb[2], x3T)
        nc.sync.dma_start(out_d.rearrange("(k p) t -> p k t", p=128), x3T[:])

    ctx.close()


# ---------------------------------------------------------------- host side
_PROGRAM = None


def _get_program():
    global _PROGRAM
    if _PROGRAM is None:
        _PROGRAM = build_program()
    return _PROGRAM


def _build_cmask(parity):
    qb = EV if parity == 0 else OD
    m = np.zeros((NKT, 128, 128), np.float32)
    tri = np.zeros((128, 128), np.float32)
    kk, qq = np.meshgrid(np.arange(128), np.arange(128), indexing="ij")
    tri[qq < kk] = MASK_NEG
    for kt in range(NKT):
        gk = KB[kt]
        gq = qb[J0U[kt]]
        if gk == gq:
            m[kt] = tri
        elif gk > gq:
            m[kt] = MASK_NEG
    return m


def kernel(**inputs):
    nc = _get_program()
    x = np.asarray(inputs["x"], np.float32)
    ctx_in = np.asarray(inputs["ctx"], np.float32)
    weights = {name: np.ascontiguousarray(np.asarray(inputs[name], np.float32))
               for name, _ in WEIGHT_SPECS}
    in_maps = []
    shard_info = []
    for c in range(N_CORES):
        b, p = c // 2, c % 2
        blocks = EV if p == 0 else OD
        tok = np.concatenate([np.arange(g * 128, (g + 1) * 128) for g in blocks])
        m = {
            "xt": np.ascontiguousarray(x[b][tok].T),
            "ctxt": np.ascontiguousarray(ctx_in[b][tok].T),
            "cmask": _build_cmask(p),
        }
        m.update(weights)
        in_maps.append(m)
        shard_info.append((b, tok))
    res = run_bass_kernel_spmd(nc, in_maps, list(range(N_CORES)))
    out = np.zeros((B, S, D), np.float32)
    for c in range(N_CORES):
        b, tok = shard_info[c]
        out[b, tok, :] = res.results[c]["out"].T
    return out


# revision 13
# speedup vs baseline: 1.2900x; 1.2900x over previous
"""Trainium2 Bass kernel for a GPT2-style decoder block (causal self-attn +
cross-attn + MLP, post-LN residuals).

Sharding: 8 cores = 4 pairs. Pair p handles batch element p (data parallel over
B=4); within a pair the 1024 tokens are split 512/512 by interleaved 128-blocks
([0,3,4,7] / [1,2,5,6]) so causal-attention work is balanced. K/V for both
attentions are exchanged inside each pair with an AllGather. Activations are
kept feature-major ("T layout": features on partitions, tokens on free) so every
matmul contraction runs over the partition axis; LayerNorm statistics are
computed with ones-vector matmuls on the PE, and the softmax denominator comes
free from a ones-column appended to V.

The SPMD program is identical on all cores; the parity-dependent causal
structure enters only through data (the host-computed `cmask` tensor and the
sharded inputs).
"""
import sys

sys.path.insert(0, '/opt/trn_rl_repo')

import contextlib

import numpy as np
import concourse.bacc as bacc
import concourse.mybir as mybir
import concourse.tile as tile
from concourse.bass_utils import run_bass_kernel_spmd

# ---------------------------------------------------------------- constants
B, S, D, H, HD, FF = 4, 1024, 1024, 16, 64, 4096
SH = 512                 # tokens per core
KD = D // 128            # 8 feature tiles of the model dim
KF = FF // 128           # 32 feature tiles of the MLP hidden dim
NKT = S // 128           # 8 key tiles (full sequence)
EV = [0, 3, 4, 7]        # query blocks of even cores (member 0 of each pair)
OD = [1, 2, 5, 6]        # query blocks of odd cores
KB = EV + OD             # key-tile order in the gathered K/V buffers
J0U = [0, 1, 2, 3, 0, 1, 2, 3]   # first computed q-block per key tile (union)
LN_EPS = 1e-5
MASK_NEG = -1e5
N_CORES = 8

f32 = mybir.dt.float32
f32r = mybir.dt.float32r
AF = mybir.ActivationFunctionType
ALU = mybir.AluOpType

# kv exchange buffer: k section [8, 128, 512] (feature-tile major), then
# v section [4, 128, 1024] (token-block major)
KV_K_ELEMS = NKT * 128 * SH
KV_V_ELEMS = 4 * 128 * D
KV_ELEMS = KV_K_ELEMS + KV_V_ELEMS

WEIGHT_SPECS = [
    ("c_attn_w", [D, 3 * D]), ("c_attn_b", [3 * D]),
    ("self_proj_w", [D, D]), ("self_proj_b", [D]),
    ("q_w", [D, D]), ("q_b", [D]),
    ("kv_w", [D, 2 * D]), ("kv_b", [2 * D]),
    ("cross_proj_w", [D, D]), ("cross_proj_b", [D]),
    ("fc_w", [D, FF]), ("fc_b", [FF]),
    ("mlp_proj_w", [FF, D]), ("mlp_proj_b", [D]),
    ("ln1_g", [D]), ("ln1_b", [D]), ("ln2_g", [D]), ("ln2_b", [D]),
    ("ln3_g", [D]), ("ln3_b", [D]),
]
BIAS_NAMES = [n for n, s in WEIGHT_SPECS if len(s) == 1]

# packed weight strips: [n, 128, 512], one contiguous 256KB block per
# (m-chunk-of-4, k) stationary strip, in exact consumption order
PACKED_SPECS = [
    ("pw_qk", 32), ("pw_vs", 16), ("pw_sp", 16), ("pw_q", 16),
    ("pw_kck", 16), ("pw_kcv", 16), ("pw_cp", 16), ("pw_fc", 64),
    ("pw_mp", 64),
]


def build_program():
    nc = bacc.Bacc("TRN2", target_bir_lowering=False, debug=False,
                   num_devices=N_CORES)

    xt_d = nc.dram_tensor("xt", [D, SH], f32, kind="ExternalInput").ap()
    ctxt_d = nc.dram_tensor("ctxt", [D, SH], f32, kind="ExternalInput").ap()
    cmask_d = nc.dram_tensor("cmask", [NKT, 128, 128], f32,
                             kind="ExternalInput").ap()
    esel_d = nc.dram_tensor("esel", [KD, H, 128], f32, kind="ExternalInput").ap()
    w = {}
    for name in BIAS_NAMES:
        w[name] = nc.dram_tensor(name, [dict(WEIGHT_SPECS)[name][0]], f32,
                                 kind="ExternalInput").ap()
    for name, n in PACKED_SPECS:
        w[name] = nc.dram_tensor(name, [n, 128, 512], f32,
                                 kind="ExternalInput").ap()
    out_d = nc.dram_tensor("out", [D, SH], f32, kind="ExternalOutput").ap()

    with tile.TileContext(nc) as tc:
        build_body(nc, tc, xt_d, ctxt_d, cmask_d, esel_d, w, out_d)
    nc.compile()
    return nc


def build_body(nc, tc, xt_d, ctxt_d, cmask_d, esel_d, w, out_d):
    ctx = contextlib.ExitStack()
    const = ctx.enter_context(tc.tile_pool(name="const", bufs=1))
    wp = ctx.enter_context(tc.tile_pool(name="wp", bufs=4))
    resid = ctx.enter_context(tc.tile_pool(name="resid", bufs=2))
    scr = ctx.enter_context(tc.tile_pool(name="scr", bufs=3))
    small = ctx.enter_context(tc.tile_pool(name="small", bufs=2))
    mm = ctx.enter_context(tc.tile_pool(name="mm", bufs=6, space="PSUM"))
    aux = ctx.enter_context(tc.tile_pool(name="aux", bufs=2, space="PSUM"))
    dram = ctx.enter_context(tc.tile_pool(name="dram", bufs=1, space="DRAM"))

    # ---------------- constants in SBUF
    def load_col_pack(vec_ap, n_tiles, name):
        """[N] vector -> [128, N/128] (column m = bias slice for feature tile m)"""
        t = const.tile([128, n_tiles], f32, tag=name, name=name)
        nc.sync.dma_start(t[:], vec_ap.rearrange("(t p) -> p t", p=128))
        return t

    cab = load_col_pack(w["c_attn_b"][0:2 * D], 16, "cab")       # q,k biases
    spb = load_col_pack(w["self_proj_b"], KD, "spb")
    qb = load_col_pack(w["q_b"], KD, "qb")
    kvbk = load_col_pack(w["kv_b"][0:D], KD, "kvbk")
    cpb = load_col_pack(w["cross_proj_b"], KD, "cpb")
    fcb = load_col_pack(w["fc_b"], KF, "fcb")
    mpb = load_col_pack(w["mlp_proj_b"], KD, "mpb")
    lng = [load_col_pack(w[f"ln{i}_g"], KD, f"ln{i}g") for i in (1, 2, 3)]
    lnb = [load_col_pack(w[f"ln{i}_b"], KD, f"ln{i}b") for i in (1, 2, 3)]

    def load_row_bcast(vec_ap, name):
        """[D] vector -> [128, D] with the vector replicated on every partition"""
        row = const.tile([1, D], f32, tag="vbrow", name=name + "row")
        nc.sync.dma_start(row[:], vec_ap[None, :])
        t = const.tile([128, D], f32, tag="vb" + name, name=name)
        nc.gpsimd.partition_broadcast(t[:], row[:])
        return t

    vb_self = load_row_bcast(w["c_attn_b"][2 * D:3 * D], "vbs")
    vb_cross = load_row_bcast(w["kv_b"][D:2 * D], "vbc")

    cmask_sb = const.tile([128, NKT, 128], f32, tag="cmask")
    nc.sync.dma_start(cmask_sb[:], cmask_d.transpose([1, 0, 2]))
    esel_sb = const.tile([H, KD, 128], f32r, tag="esel")
    nc.sync.dma_start(esel_sb[:], esel_d.transpose([1, 0, 2]).bitcast(f32r))

    ones_f = const.tile([128, 128], f32, tag="onesf")
    nc.vector.memset(ones_f[:], 1.0)
    eps_sb = const.tile([128, 1], f32, tag="eps")
    nc.vector.memset(eps_sb[:], LN_EPS)
    ones_mm = const.tile([128, 1], f32r, tag="onesmm")
    nc.vector.tensor_copy(ones_mm[:], ones_f[:, 0:1])
    ones_row = const.tile([1, 128], f32r, tag="onesrow")
    nc.vector.tensor_copy(ones_row[:], ones_f[0:1, :])

    # input activations, feature-major
    xT = resid.tile([128, KD, SH], f32r, tag="resid")
    nc.sync.dma_start(xT[:], xt_d.rearrange("(k p) t -> p k t", p=128).bitcast(f32r))

    # ---------------- generic building blocks
    def wstrip(pw, idx):
        t = wp.tile([128, 512], f32r, tag="w", name="w")
        nc.sync.dma_start(t[:], pw[idx].bitcast(f32r))
        return t

    def linearT(src, pw, mt, kt, consume):
        """feature-major out tiles m in [0,mt): psum_m = sum_k strip(m,k).T @ src(k)"""
        for mc in range(0, mt, 4):
            nchunk = min(4, mt - mc)
            psums = [mm.tile([128, SH], f32, tag="mm", name=f"ps{i}")
                     for i in range(nchunk)]
            for k in range(kt):
                ws = wstrip(pw, (mc // 4) * kt + k)
                for i in range(nchunk):
                    nc.tensor.matmul(psums[i][:], ws[:, i * 128:(i + 1) * 128],
                                     src(k), start=(k == 0), stop=(k == kt - 1))
            for i in range(nchunk):
                consume(mc + i, psums[i])

    def layer_norm(rT, g_sb, b_sb, outT):
        """outT = LN(rT) * g + b; stats over features via ones-matmuls on PE."""
        ps_sum = aux.tile([1, SH], f32, tag="aux", name="ps_sum")
        ps_sq = aux.tile([1, SH], f32, tag="aux", name="ps_sq")
        for k in range(KD):
            nc.tensor.matmul(ps_sum[:], ones_mm[:], rT[:, k, :],
                             start=(k == 0), stop=(k == KD - 1))
        for k in range(KD):
            sq = scr.tile([128, SH], f32r, tag="sq", name="sq")
            nc.scalar.activation(sq[:], rT[:, k, :], AF.Square)
            nc.tensor.matmul(ps_sq[:], ones_mm[:], sq[:],
                             start=(k == 0), stop=(k == KD - 1))
        mean = small.tile([1, SH], f32r, tag="stat", bufs=4, name="mean")
        nc.vector.tensor_scalar_mul(mean[:], ps_sum[:], 1.0 / D)
        var = small.tile([1, SH], f32, tag="stat", bufs=4, name="var")
        # var = ps_sq/D - mean^2, computed as (mean * -mean) + ps_sq/D
        nc.vector.scalar_tensor_tensor(var[:], mean[:], -1.0, mean[:],
                                       op0=ALU.mult, op1=ALU.mult)
        nc.vector.scalar_tensor_tensor(var[:], ps_sq[:], 1.0 / D, var[:],
                                       op0=ALU.mult, op1=ALU.add)
        sd = small.tile([1, SH], f32, tag="stat", bufs=4, name="sd")
        nc.scalar.activation(sd[:], var[:], AF.Sqrt, bias=eps_sb[0:1, :])
        rstd = small.tile([1, SH], f32r, tag="stat", bufs=4, name="rstd")
        with nc.allow_low_precision(reason="LN rstd in fp32r"):
            nc.vector.reciprocal(rstd[:], sd[:])
        # broadcast mean/rstd across partitions on the PE (K=1 matmuls)
        mb_ps = mm.tile([128, SH], f32, tag="mm", name="mb_ps")
        rs_ps = mm.tile([128, SH], f32, tag="mm", name="rs_ps")
        nc.tensor.matmul(mb_ps[:], ones_row[:], mean[:], start=True, stop=True)
        nc.tensor.matmul(rs_ps[:], ones_row[:], rstd[:], start=True, stop=True)
        for k in range(KD):
            t = scr.tile([128, SH], f32, tag="lnt", name="lnt")
            nc.vector.tensor_sub(t[:], rT[:, k, :], mb_ps[:])
            nc.vector.tensor_mul(t[:], t[:], rs_ps[:])
            nc.vector.tensor_scalar(outT[:, k, :], t[:],
                                    scalar1=g_sb[:, k:k + 1],
                                    scalar2=b_sb[:, k:k + 1],
                                    op0=ALU.mult, op1=ALU.add)

    def proj_resid_ln(aT_sb, pw, bias_sb, xin, g_sb, b_sb, xout):
        rT = resid.tile([128, KD, SH], f32r, tag="resid", name="rT")

        def consume(m, psum):
            nc.vector.scalar_tensor_tensor(rT[:, m, :], psum[:],
                                           bias_sb[:, m:m + 1], xin[:, m, :],
                                           op0=ALU.add, op1=ALU.add)

        linearT(lambda k: aT_sb[:, k, :], pw, KD, KD, consume)
        layer_norm(rT, g_sb, b_sb, xout)

    # =================================================== attention phases
    with tc.tile_pool(name="attn", bufs=1) as ap:

        def kv_produce(srcT, pwk, pwv, kbias, vb_bcast, kv_in):
            """K (feature-major) + V (token-major) for this core's 512 tokens,
            written to the kv_in DRAM buffer for the pair AllGather."""
            kv_flat = kv_in[:]

            def consume_k(m, psum):
                ks = ap.tile([128, SH], f32r, tag="kstage", bufs=2, name="ks")
                nc.vector.tensor_scalar_add(ks[:], psum[:], kbias[:, m:m + 1])
                dst = kv_flat[m * 128 * SH:(m + 1) * 128 * SH]
                nc.sync.dma_start(dst.rearrange("(p t) -> p t", p=128),
                                  ks[:].bitcast(f32))

            linearT(lambda k: srcT[:, k, :], pwk, KD, KD, consume_k)

            # V token-stationary: psum [128 tokens, 512 v-cols]
            v_dst = kv_flat[KV_K_ELEMS:].rearrange("(b p e) -> b p e",
                                                   p=128, e=D)
            for vc in range(2):
                psums = [mm.tile([128, SH], f32, tag="mm", name=f"psv{i}")
                         for i in range(4)]
                for k in range(KD):
                    ws = wstrip(pwv, vc * KD + k)
                    for tb in range(4):
                        nc.tensor.matmul(psums[tb][:],
                                         srcT[:, k, tb * 128:(tb + 1) * 128],
                                         ws[:], start=(k == 0),
                                         stop=(k == KD - 1))
                for tb in range(4):
                    vs = ap.tile([128, SH], f32r, tag="vstage", bufs=2,
                                 name="vs")
                    nc.vector.tensor_add(vs[:], psums[tb][:],
                                         vb_bcast[:, vc * 512:(vc + 1) * 512])
                    nc.sync.dma_start(v_dst[tb, :, vc * 512:(vc + 1) * 512],
                                      vs[:].bitcast(f32))

        def kv_gather(kv_in, tag):
            kv_out = dram.tile([2, KV_ELEMS], f32, tag="kvout" + tag,
                               name="kvout" + tag)
            nc.gpsimd.collective_compute(
                "AllGather", ALU.bypass,
                replica_groups=[[0, 1], [2, 3], [4, 5], [6, 7]],
                ins=[kv_in.opt()],
                outs=[kv_out.opt()],
            )
            return kv_out

        def load_v(kv_out, name):
            """gathered V -> v_sb flat [128, H*NKT*65], ones in column 65."""
            v_sb = ap.tile([128, H * NKT * 65], f32r, tag="v", name=name)
            v4 = v_sb[:].rearrange("p (h kt e) -> p h kt e", kt=NKT, e=65)
            nc.vector.tensor_copy(
                v_sb[:].rearrange("p (c e) -> p c e", e=65)[:, :, 64],
                ones_f[:])
            for mem in range(2):
                vsec = kv_out[mem, KV_K_ELEMS:].rearrange(
                    "(b p h e) -> b p h e", p=128, h=H, e=HD)
                for tb in range(4):
                    nc.sync.dma_start(v4[:, :, mem * 4 + tb, 0:64],
                                      vsec[tb].bitcast(f32r))
            return v_sb

        def attention(qT_sb, kv_out, v_sb, aT_sb, masked):
            csum = ap.tile([H, SH], f32, tag="csum", name="csum")
            for hp in range(H // 2):
                kp = ap.tile([128, S], f32r, tag="kp", bufs=2, name="kp")
                for mem in range(2):
                    ksec = kv_out[mem, 0:KV_K_ELEMS].rearrange(
                        "(m p t) -> p m t", p=128, t=SH)
                    nc.sync.dma_start(kp[:, mem * SH:(mem + 1) * SH],
                                      ksec[:, hp, :].bitcast(f32r))
                for hh in range(2):
                    h = 2 * hp + hh
                    hr = hh * 64
                    ps_av = aux.tile([65, SH], f32, tag="aux", name="ps_av")
                    for kt in range(NKT):
                        j0 = J0U[kt] if masked else 0
                        q0 = j0 * 128
                        qn = SH - q0
                        ps_s = mm.tile([128, SH], f32, tag="mm", name="ps_s")
                        nc.tensor.matmul(
                            ps_s[:, 0:qn],
                            kp[hr:hr + 64, kt * 128:(kt + 1) * 128],
                            qT_sb[hr:hr + 64, hp, q0:SH],
                            start=True, stop=True)
                        if masked:
                            nc.vector.tensor_add(ps_s[:, 0:128],
                                                 ps_s[:, 0:128],
                                                 cmask_sb[:, kt, :])
                        pr = ap.tile([128, SH], f32r, tag="probs", bufs=4,
                                     name="pr")
                        nc.scalar.activation(pr[:, 0:qn], ps_s[:, 0:qn],
                                             AF.Exp, bias=0.0, scale=0.125)
                        nc.tensor.matmul(
                            ps_av[:, q0:SH],
                            v_sb[:, (h * NKT + kt) * 65:(h * NKT + kt) * 65 + 65],
                            pr[:, 0:qn],
                            start=(kt == 0), stop=(kt == NKT - 1))
                    # raw av -> aT; colsum row -> csum[h]
                    ast = ap.tile([65, SH], f32r, tag="astage", bufs=3,
                                  name="ast")
                    nc.vector.tensor_copy(ast[:], ps_av[:])
                    nc.sync.dma_start(aT_sb[hr:hr + 64, hp, :], ast[0:64, :])
                    nc.sync.dma_start(csum[h:h + 1, :],
                                      ast[64:65, :].bitcast(f32))
            # one reciprocal for all heads, then normalize aT per head pair
            rec = ap.tile([H, SH], f32r, tag="rec", name="rec")
            with nc.allow_low_precision(reason="softmax denom in fp32r"):
                nc.vector.reciprocal(rec[:], csum[:])
            for m in range(KD):
                rb_ps = mm.tile([128, SH], f32, tag="mm", name="rb_ps")
                nc.tensor.matmul(rb_ps[:], esel_sb[:, m, :], rec[:],
                                 start=True, stop=True)
                nc.vector.tensor_mul(aT_sb[:, m, :], aT_sb[:, m, :], rb_ps[:])

        # ---------- kv production + gathers first (hide collective latency)
        kv_in_s = dram.tile([KV_ELEMS], f32, tag="kvins", name="kvins")
        kv_produce(xT, w["pw_qk"][16:32], w["pw_vs"],
                   kbias=cab[:, 8:16], vb_bcast=vb_self, kv_in=kv_in_s)
        kv_out_s = kv_gather(kv_in_s, "s")

        ctxT = ap.tile([128, KD, SH], f32r, tag="ctxT", name="ctxT")
        nc.sync.dma_start(
            ctxT[:], ctxt_d.rearrange("(k p) t -> p k t", p=128).bitcast(f32r))
        kv_in_c = dram.tile([KV_ELEMS], f32, tag="kvinc", name="kvinc")
        kv_produce(ctxT, w["pw_kck"], w["pw_kcv"],
                   kbias=kvbk, vb_bcast=vb_cross, kv_in=kv_in_c)
        kv_out_c = kv_gather(kv_in_c, "c")

        # ---------- phase 1: causal self-attention
        qT = ap.tile([128, KD, SH], f32r, tag="qT", bufs=1, name="qT")

        def consume_q(m, psum):
            nc.vector.tensor_scalar_add(qT[:, m, :], psum[:], cab[:, m:m + 1])

        linearT(lambda k: xT[:, k, :], w["pw_qk"], KD, KD, consume_q)

        v_s = load_v(kv_out_s, "v_s")
        aT = ap.tile([128, KD, SH], f32r, tag="aT", bufs=1, name="aT")
        attention(qT, kv_out_s, v_s, aT, masked=True)

        x1T = resid.tile([128, KD, SH], f32r, tag="resid", name="x1T")
        proj_resid_ln(aT, w["pw_sp"], spb, xT, lng[0], lnb[0], x1T)

        # ---------- phase 2: cross-attention
        q1T = ap.tile([128, KD, SH], f32r, tag="qT", bufs=1, name="q1T")

        def consume_q1(m, psum):
            nc.vector.tensor_scalar_add(q1T[:, m, :], psum[:], qb[:, m:m + 1])

        linearT(lambda k: x1T[:, k, :], w["pw_q"], KD, KD, consume_q1)

        v_c = load_v(kv_out_c, "v_c")
        a2T = ap.tile([128, KD, SH], f32r, tag="aT", bufs=1, name="a2T")
        attention(q1T, kv_out_c, v_c, a2T, masked=False)

        x2T = resid.tile([128, KD, SH], f32r, tag="resid", name="x2T")
        proj_resid_ln(a2T, w["pw_cp"], cpb, x1T, lng[1], lnb[1], x2T)

    # =================================================== phase 3: MLP
    with tc.tile_pool(name="mlp", bufs=1) as mp:
        h_sb = mp.tile([128, KF, SH], f32r, tag="h", name="h_sb")

        def consume_h(m, psum):
            nc.scalar.activation(h_sb[:, m, :], psum[:], AF.Gelu_apprx_tanh,
                                 bias=fcb[:, m:m + 1], scale=1.0)

        linearT(lambda k: x2T[:, k, :], w["pw_fc"], KF, KD, consume_h)

        r3T = resid.tile([128, KD, SH], f32r, tag="resid", name="r3T")

        def consume_m(m, psum):
            nc.vector.scalar_tensor_tensor(r3T[:, m, :], psum[:],
                                           mpb[:, m:m + 1], x2T[:, m, :],
                                           op0=ALU.add, op1=ALU.add)

        linearT(lambda k: h_sb[:, k, :], w["pw_mp"], KD, KF, consume_m)

        x3T = resid.tile([128, KD, SH], f32, tag="resid", name="x3T")
        layer_norm(r3T, lng[2], lnb[2], x3T)
        nc.sync.dma_start(out_d.rearrange("(k p) t -> p k t", p=128), x3T[:])

    ctx.close()


# ---------------------------------------------------------------- host side
_PROGRAM = None
_PACKED = None


def _get_program():
    global _PROGRAM
    if _PROGRAM is None:
        _PROGRAM = build_program()
    return _PROGRAM


def _build_cmask(parity):
    qb = EV if parity == 0 else OD
    m = np.zeros((NKT, 128, 128), np.float32)
    tri = np.zeros((128, 128), np.float32)
    kk, qq = np.meshgrid(np.arange(128), np.arange(128), indexing="ij")
    tri[qq < kk] = MASK_NEG
    for kt in range(NKT):
        gk = KB[kt]
        gq = qb[J0U[kt]]
        if gk == gq:
            m[kt] = tri
        elif gk > gq:
            m[kt] = MASK_NEG
    return m


def _pack_lin(wm, m0, mt, kt):
    """strips in linearT consumption order: mc-chunk-of-4 outer, k inner."""
    out = np.empty(((mt // 4) * kt, 128, 512), np.float32)
    i = 0
    for mc in range(0, mt, 4):
        for k in range(kt):
            out[i] = wm[k * 128:(k + 1) * 128, (m0 + mc) * 128:(m0 + mc + 4) * 128]
            i += 1
    return out


def _pack_v(wm, vcol0):
    """strips in v-loop order: vc outer, k inner."""
    out = np.empty((2 * KD, 128, 512), np.float32)
    i = 0
    for vc in range(2):
        for k in range(KD):
            out[i] = wm[k * 128:(k + 1) * 128, vcol0 + vc * 512:vcol0 + (vc + 1) * 512]
            i += 1
    return out


def _pack_weights(inputs):
    f = lambda n: np.asarray(inputs[n], np.float32)
    packed = {
        "pw_qk": _pack_lin(f("c_attn_w"), 0, 16, KD),
        "pw_vs": _pack_v(f("c_attn_w"), 2 * D),
        "pw_sp": _pack_lin(f("self_proj_w"), 0, KD, KD),
        "pw_q": _pack_lin(f("q_w"), 0, KD, KD),
        "pw_kck": _pack_lin(f("kv_w"), 0, KD, KD),
        "pw_kcv": _pack_v(f("kv_w"), D),
        "pw_cp": _pack_lin(f("cross_proj_w"), 0, KD, KD),
        "pw_fc": _pack_lin(f("fc_w"), 0, KF, KD),
        "pw_mp": _pack_lin(f("mlp_proj_w"), 0, KD, KF),
    }
    for name in BIAS_NAMES:
        packed[name] = np.ascontiguousarray(f(name))
    esel = np.zeros((KD, H, 128), np.float32)
    for m in range(KD):
        esel[m, 2 * m, 0:64] = 1.0
        esel[m, 2 * m + 1, 64:128] = 1.0
    packed["esel"] = esel
    return packed


def kernel(**inputs):
    nc = _get_program()
    x = np.asarray(inputs["x"], np.float32)
    ctx_in = np.asarray(inputs["ctx"], np.float32)
    packed = _pack_weights(inputs)
    in_maps = []
    shard_info = []
    for c in range(N_CORES):
        b, p = c // 2, c % 2
        blocks = EV if p == 0 else OD
        tok = np.concatenate([np.arange(g * 128, (g + 1) * 128) for g in blocks])
        m = {
            "xt": np.ascontiguousarray(x[b][tok].T),
            "ctxt": np.ascontiguousarray(ctx_in[b][tok].T),
            "cmask": _build_cmask(p),
        }
        m.update(packed)
        in_maps.append(m)
        shard_info.append((b, tok))
    res = run_bass_kernel_spmd(nc, in_maps, list(range(N_CORES)))
    out = np.zeros((B, S, D), np.float32)
    for c in range(N_CORES):
        b, tok = shard_info[c]
        out[b, tok, :] = res.results[c]["out"].T
    return out


# revision 14
# speedup vs baseline: 1.4732x; 1.1420x over previous
"""Trainium2 Bass kernel for a GPT2-style decoder block (causal self-attn +
cross-attn + MLP, post-LN residuals).

Sharding: 8 cores = 4 pairs. Pair p handles batch element p (data parallel over
B=4); within a pair the 1024 tokens are split 512/512 by interleaved 128-blocks
([0,3,4,7] / [1,2,5,6]) so causal-attention work is balanced. K/V for both
attentions are exchanged inside each pair with an AllGather. Activations are
kept feature-major ("T layout": features on partitions, tokens on free) so every
matmul contraction runs over the partition axis; LayerNorm statistics are
computed with ones-vector matmuls on the PE, and the softmax denominator comes
free from a ones-column appended to V.

The SPMD program is identical on all cores; the parity-dependent causal
structure enters only through data (the host-computed `cmask` tensor and the
sharded inputs).
"""
import sys

sys.path.insert(0, '/opt/trn_rl_repo')

import contextlib

import numpy as np
import concourse.bacc as bacc
import concourse.mybir as mybir
import concourse.tile as tile
from concourse.bass_utils import run_bass_kernel_spmd

# ---------------------------------------------------------------- constants
B, S, D, H, HD, FF = 4, 1024, 1024, 16, 64, 4096
SH = 512                 # tokens per core
KD = D // 128            # 8 feature tiles of the model dim
KF = FF // 128           # 32 feature tiles of the MLP hidden dim
NKT = S // 128           # 8 key tiles (full sequence)
EV = [0, 3, 4, 7]        # query blocks of even cores (member 0 of each pair)
OD = [1, 2, 5, 6]        # query blocks of odd cores
KB = EV + OD             # key-tile order in the gathered K/V buffers
J0U = [0, 1, 2, 3, 0, 1, 2, 3]   # first computed q-block per key tile (union)
LN_EPS = 1e-5
MASK_NEG = -1e5
N_CORES = 8

f32 = mybir.dt.float32
f32r = mybir.dt.float32r
AF = mybir.ActivationFunctionType
ALU = mybir.AluOpType

# kv exchange buffer: k section [8, 128, 512] (feature-tile major), then
# v section [4, 128, 1024] (token-block major)
KV_K_ELEMS = NKT * 128 * SH
KV_V_ELEMS = 4 * 128 * D
KV_ELEMS = KV_K_ELEMS + KV_V_ELEMS

WEIGHT_SPECS = [
    ("c_attn_w", [D, 3 * D]), ("c_attn_b", [3 * D]),
    ("self_proj_w", [D, D]), ("self_proj_b", [D]),
    ("q_w", [D, D]), ("q_b", [D]),
    ("kv_w", [D, 2 * D]), ("kv_b", [2 * D]),
    ("cross_proj_w", [D, D]), ("cross_proj_b", [D]),
    ("fc_w", [D, FF]), ("fc_b", [FF]),
    ("mlp_proj_w", [FF, D]), ("mlp_proj_b", [D]),
    ("ln1_g", [D]), ("ln1_b", [D]), ("ln2_g", [D]), ("ln2_b", [D]),
    ("ln3_g", [D]), ("ln3_b", [D]),
]
BIAS_NAMES = [n for n, s in WEIGHT_SPECS if len(s) == 1]

# packed weight strips: [n, 128, 512], one contiguous 256KB block per
# (m-chunk-of-4, k) stationary strip, in exact consumption order
PACKED_SPECS = [
    ("pw_qk", 32), ("pw_vs", 16), ("pw_sp", 16), ("pw_q", 16),
    ("pw_kck", 16), ("pw_kcv", 16), ("pw_cp", 16), ("pw_fc", 64),
    ("pw_mp", 64),
]


def build_program():
    nc = bacc.Bacc("TRN2", target_bir_lowering=False, debug=False,
                   num_devices=N_CORES)

    xt_d = nc.dram_tensor("xt", [D, S], f32, kind="ExternalInput").ap()
    ctxt_d = nc.dram_tensor("ctxt", [D, S], f32, kind="ExternalInput").ap()
    cmask_d = nc.dram_tensor("cmask", [NKT, 128, 128], f32,
                             kind="ExternalInput").ap()
    esel_d = nc.dram_tensor("esel", [KD, H, 128], f32, kind="ExternalInput").ap()
    w = {}
    for name in BIAS_NAMES:
        w[name] = nc.dram_tensor(name, [dict(WEIGHT_SPECS)[name][0]], f32,
                                 kind="ExternalInput").ap()
    for name, n in PACKED_SPECS:
        w[name] = nc.dram_tensor(name, [n, 128, 512], f32,
                                 kind="ExternalInput").ap()
    out_d = nc.dram_tensor("out", [D, SH], f32, kind="ExternalOutput").ap()

    with tile.TileContext(nc) as tc:
        build_body(nc, tc, xt_d, ctxt_d, cmask_d, esel_d, w, out_d)
    nc.compile()
    return nc


def build_body(nc, tc, xt_d, ctxt_d, cmask_d, esel_d, w, out_d):
    ctx = contextlib.ExitStack()
    const = ctx.enter_context(tc.tile_pool(name="const", bufs=1))
    wp = ctx.enter_context(tc.tile_pool(name="wp", bufs=4))
    resid = ctx.enter_context(tc.tile_pool(name="resid", bufs=2))
    scr = ctx.enter_context(tc.tile_pool(name="scr", bufs=3))
    small = ctx.enter_context(tc.tile_pool(name="small", bufs=2))
    mm = ctx.enter_context(tc.tile_pool(name="mm", bufs=6, space="PSUM"))
    aux = ctx.enter_context(tc.tile_pool(name="aux", bufs=2, space="PSUM"))
    dram = ctx.enter_context(tc.tile_pool(name="dram", bufs=1, space="DRAM"))

    # ---------------- constants in SBUF
    def load_col_pack(vec_ap, n_tiles, name):
        """[N] vector -> [128, N/128] (column m = bias slice for feature tile m)"""
        t = const.tile([128, n_tiles], f32, tag=name, name=name)
        nc.sync.dma_start(t[:], vec_ap.rearrange("(t p) -> p t", p=128))
        return t

    cab = load_col_pack(w["c_attn_b"][0:2 * D], 16, "cab")       # q,k biases
    spb = load_col_pack(w["self_proj_b"], KD, "spb")
    qb = load_col_pack(w["q_b"], KD, "qb")
    kvbk = load_col_pack(w["kv_b"][0:D], KD, "kvbk")
    cpb = load_col_pack(w["cross_proj_b"], KD, "cpb")
    fcb = load_col_pack(w["fc_b"], KF, "fcb")
    mpb = load_col_pack(w["mlp_proj_b"], KD, "mpb")
    lng = [load_col_pack(w[f"ln{i}_g"], KD, f"ln{i}g") for i in (1, 2, 3)]
    lnb = [load_col_pack(w[f"ln{i}_b"], KD, f"ln{i}b") for i in (1, 2, 3)]

    def load_row_bcast(vec_ap, name):
        """[D] vector -> [128, D] with the vector replicated on every partition"""
        row = const.tile([1, D], f32, tag="vbrow", name=name + "row")
        nc.sync.dma_start(row[:], vec_ap[None, :])
        t = const.tile([128, D], f32, tag="vb" + name, name=name)
        nc.gpsimd.partition_broadcast(t[:], row[:])
        return t

    vb_self = load_row_bcast(w["c_attn_b"][2 * D:3 * D], "vbs")
    vb_cross = load_row_bcast(w["kv_b"][D:2 * D], "vbc")

    cmask_sb = const.tile([128, NKT, 128], f32, tag="cmask")
    nc.sync.dma_start(cmask_sb[:], cmask_d.transpose([1, 0, 2]))
    esel_sb = const.tile([H, KD, 128], f32r, tag="esel")
    nc.sync.dma_start(esel_sb[:], esel_d.transpose([1, 0, 2]).bitcast(f32r))

    ones_f = const.tile([128, 128], f32, tag="onesf")
    nc.vector.memset(ones_f[:], 1.0)
    eps_sb = const.tile([128, 1], f32, tag="eps")
    nc.vector.memset(eps_sb[:], LN_EPS)
    ones_mm = const.tile([128, 1], f32r, tag="onesmm")
    nc.vector.tensor_copy(ones_mm[:], ones_f[:, 0:1])
    ones_row = const.tile([1, 128], f32r, tag="onesrow")
    nc.vector.tensor_copy(ones_row[:], ones_f[0:1, :])

    # input activations, feature-major
    xT = resid.tile([128, KD, SH], f32r, tag="resid")
    nc.sync.dma_start(
        xT[:], xt_d.rearrange("(k p) t -> p k t", p=128)[:, :, 0:SH].bitcast(f32r))

    # ---------------- generic building blocks
    def wstrip(pw, idx):
        t = wp.tile([128, 512], f32r, tag="w", name="w")
        nc.sync.dma_start(t[:], pw[idx].bitcast(f32r))
        return t

    def linearT(src, pw, mt, kt, consume):
        """feature-major out tiles m in [0,mt): psum_m = sum_k strip(m,k).T @ src(k)"""
        for mc in range(0, mt, 4):
            nchunk = min(4, mt - mc)
            psums = [mm.tile([128, SH], f32, tag="mm", name=f"ps{i}")
                     for i in range(nchunk)]
            for k in range(kt):
                ws = wstrip(pw, (mc // 4) * kt + k)
                for i in range(nchunk):
                    nc.tensor.matmul(psums[i][:], ws[:, i * 128:(i + 1) * 128],
                                     src(k), start=(k == 0), stop=(k == kt - 1))
            for i in range(nchunk):
                consume(mc + i, psums[i])

    def layer_norm(rT, g_sb, b_sb, outT):
        """outT = LN(rT) * g + b; stats over features via ones-matmuls on PE."""
        ps_sum = aux.tile([1, SH], f32, tag="aux", name="ps_sum")
        ps_sq = aux.tile([1, SH], f32, tag="aux", name="ps_sq")
        for k in range(KD):
            nc.tensor.matmul(ps_sum[:], ones_mm[:], rT[:, k, :],
                             start=(k == 0), stop=(k == KD - 1))
        for k in range(KD):
            sq = scr.tile([128, SH], f32r, tag="sq", name="sq")
            nc.scalar.activation(sq[:], rT[:, k, :], AF.Square)
            nc.tensor.matmul(ps_sq[:], ones_mm[:], sq[:],
                             start=(k == 0), stop=(k == KD - 1))
        mean = small.tile([1, SH], f32r, tag="stat", bufs=4, name="mean")
        nc.vector.tensor_scalar_mul(mean[:], ps_sum[:], 1.0 / D)
        var = small.tile([1, SH], f32, tag="stat", bufs=4, name="var")
        # var = ps_sq/D - mean^2, computed as (mean * -mean) + ps_sq/D
        nc.vector.scalar_tensor_tensor(var[:], mean[:], -1.0, mean[:],
                                       op0=ALU.mult, op1=ALU.mult)
        nc.vector.scalar_tensor_tensor(var[:], ps_sq[:], 1.0 / D, var[:],
                                       op0=ALU.mult, op1=ALU.add)
        sd = small.tile([1, SH], f32, tag="stat", bufs=4, name="sd")
        nc.scalar.activation(sd[:], var[:], AF.Sqrt, bias=eps_sb[0:1, :])
        rstd = small.tile([1, SH], f32r, tag="stat", bufs=4, name="rstd")
        with nc.allow_low_precision(reason="LN rstd in fp32r"):
            nc.vector.reciprocal(rstd[:], sd[:])
        # broadcast mean/rstd across partitions on the PE (K=1 matmuls)
        mb_ps = mm.tile([128, SH], f32, tag="mm", name="mb_ps")
        rs_ps = mm.tile([128, SH], f32, tag="mm", name="rs_ps")
        nc.tensor.matmul(mb_ps[:], ones_row[:], mean[:], start=True, stop=True)
        nc.tensor.matmul(rs_ps[:], ones_row[:], rstd[:], start=True, stop=True)
        for k in range(KD):
            t = scr.tile([128, SH], f32, tag="lnt", name="lnt")
            nc.vector.tensor_sub(t[:], rT[:, k, :], mb_ps[:])
            nc.vector.tensor_mul(t[:], t[:], rs_ps[:])
            nc.vector.tensor_scalar(outT[:, k, :], t[:],
                                    scalar1=g_sb[:, k:k + 1],
                                    scalar2=b_sb[:, k:k + 1],
                                    op0=ALU.mult, op1=ALU.add)

    def proj_resid_ln(aT_sb, pw, bias_sb, xin, g_sb, b_sb, xout):
        rT = resid.tile([128, KD, SH], f32r, tag="resid", name="rT")

        def consume(m, psum):
            nc.vector.scalar_tensor_tensor(rT[:, m, :], psum[:],
                                           bias_sb[:, m:m + 1], xin[:, m, :],
                                           op0=ALU.add, op1=ALU.add)

        linearT(lambda k: aT_sb[:, k, :], pw, KD, KD, consume)
        layer_norm(rT, g_sb, b_sb, xout)

    # =================================================== attention phases
    # All of K/V is computed locally from the replicated full x / ctx
    # (no collectives); K spills to DRAM and streams back per head pair,
    # V accumulates straight into SBUF.
    with tc.tile_pool(name="attn", bufs=1) as ap:

        def load_chunk(src_d, c, name):
            t = ap.tile([128, KD, SH], f32r, tag="src", bufs=1, name=name)
            nc.sync.dma_start(
                t[:], src_d.rearrange("(k p) t -> p k t", p=128)
                [:, :, c * SH:(c + 1) * SH].bitcast(f32r))
            return t

        def k_produce(xf, c, pwk, kbias, kbuf):
            def consume_k(m, psum):
                ks = ap.tile([128, SH], f32r, tag="kstage", bufs=2, name="ks")
                nc.vector.tensor_scalar_add(ks[:], psum[:], kbias[:, m:m + 1])
                nc.scalar.dma_start(kbuf[m, :, c * SH:(c + 1) * SH],
                                    ks[:].bitcast(f32))

            linearT(lambda k: xf[:, k, :], pwk, KD, KD, consume_k)

        def v_produce(xf, tbc, pwv, vb_bcast, v_sb):
            v4 = v_sb[:].rearrange("p (h kt e) -> p h kt e", kt=NKT, e=65)
            for vc in range(2):
                psums = [mm.tile([128, SH], f32, tag="mm", name=f"psv{i}")
                         for i in range(4)]
                for k in range(KD):
                    ws = wstrip(pwv, vc * KD + k)
                    for i in range(4):
                        nc.tensor.matmul(
                            psums[i][:], xf[:, k, i * 128:(i + 1) * 128],
                            ws[:], start=(k == 0), stop=(k == KD - 1))
                for i in range(4):
                    nc.vector.tensor_add(
                        v4[:, vc * 8:(vc + 1) * 8, tbc * 4 + i, 0:64],
                        psums[i][:].rearrange("p (h e) -> p h e", e=HD),
                        vb_bcast[:, vc * 512:(vc + 1) * 512]
                        .rearrange("p (h e) -> p h e", e=HD))

        def make_v(name):
            v_sb = ap.tile([128, H * NKT * 65], f32r, tag="v", name=name)
            nc.vector.tensor_copy(
                v_sb[:].rearrange("p (c e) -> p c e", e=65)[:, :, 64],
                ones_f[:])
            return v_sb

        def attention(qT_sb, kbuf, v_sb, aT_sb, masked):
            csum = ap.tile([H, SH], f32, tag="csum", name="csum")
            for hp in range(H // 2):
                kp = ap.tile([128, S], f32r, tag="kp", bufs=2, name="kp")
                nc.scalar.dma_start(kp[:], kbuf[hp].bitcast(f32r))
                for hh in range(2):
                    h = 2 * hp + hh
                    hr = hh * 64
                    ps_av = aux.tile([65, SH], f32, tag="aux", name="ps_av")
                    for kt in range(NKT):
                        j0 = J0U[kt] if masked else 0
                        q0 = j0 * 128
                        qn = SH - q0
                        ps_s = mm.tile([128, SH], f32, tag="mm", name="ps_s")
                        nc.tensor.matmul(
                            ps_s[:, 0:qn],
                            kp[hr:hr + 64, kt * 128:(kt + 1) * 128],
                            qT_sb[hr:hr + 64, hp, q0:SH],
                            start=True, stop=True)
                        if masked:
                            nc.vector.tensor_add(ps_s[:, 0:128],
                                                 ps_s[:, 0:128],
                                                 cmask_sb[:, kt, :])
                        pr = ap.tile([128, SH], f32r, tag="probs", bufs=4,
                                     name="pr")
                        nc.scalar.activation(pr[:, 0:qn], ps_s[:, 0:qn],
                                             AF.Exp, bias=0.0, scale=0.125)
                        nc.tensor.matmul(
                            ps_av[:, q0:SH],
                            v_sb[:, (h * NKT + kt) * 65:(h * NKT + kt) * 65 + 65],
                            pr[:, 0:qn],
                            start=(kt == 0), stop=(kt == NKT - 1))
                    # raw av -> aT; colsum row -> csum[h]
                    ast = ap.tile([65, SH], f32r, tag="astage", bufs=2,
                                  name="ast")
                    nc.vector.tensor_copy(ast[:], ps_av[:])
                    nc.sync.dma_start(aT_sb[hr:hr + 64, hp, :], ast[0:64, :])
                    nc.sync.dma_start(csum[h:h + 1, :],
                                      ast[64:65, :].bitcast(f32))
            # one reciprocal for all heads, then normalize aT per head pair
            rec = ap.tile([H, SH], f32r, tag="rec", name="rec")
            with nc.allow_low_precision(reason="softmax denom in fp32r"):
                nc.vector.reciprocal(rec[:], csum[:])
            for m in range(KD):
                rb_ps = mm.tile([128, SH], f32, tag="mm", name="rb_ps")
                nc.tensor.matmul(rb_ps[:], esel_sb[:, m, :], rec[:],
                                 start=True, stop=True)
                nc.vector.tensor_mul(aT_sb[:, m, :], aT_sb[:, m, :], rb_ps[:])

        vb_self = load_row_bcast(w["c_attn_b"][2 * D:3 * D], "vbs")
        vb_cross = load_row_bcast(w["kv_b"][D:2 * D], "vbc")

        # ---------- phase 1: causal self-attention
        qT = ap.tile([128, KD, SH], f32r, tag="qT", bufs=1, name="qT")
        kbuf_s = dram.tile([KD, 128, S], f32, tag="kbufs", name="kbufs")
        v_s = make_v("v_s")

        def consume_q(m, psum):
            nc.vector.tensor_scalar_add(qT[:, m, :], psum[:], cab[:, m:m + 1])

        xf0 = load_chunk(xt_d, 0, "xf0")
        linearT(lambda k: xf0[:, k, :], w["pw_qk"], KD, KD, consume_q)
        k_produce(xf0, 0, w["pw_qk"][16:32], cab[:, 8:16], kbuf_s)
        v_produce(xf0, 0, w["pw_vs"], vb_self, v_s)
        xf1 = load_chunk(xt_d, 1, "xf1")
        k_produce(xf1, 1, w["pw_qk"][16:32], cab[:, 8:16], kbuf_s)
        v_produce(xf1, 1, w["pw_vs"], vb_self, v_s)

        aT = ap.tile([128, KD, SH], f32r, tag="aT", bufs=1, name="aT")
        attention(qT, kbuf_s, v_s, aT, masked=True)

        x1T = resid.tile([128, KD, SH], f32r, tag="resid", name="x1T")
        proj_resid_ln(aT, w["pw_sp"], spb, xT, lng[0], lnb[0], x1T)

        # ---------- phase 2: cross-attention
        kbuf_c = dram.tile([KD, 128, S], f32, tag="kbufc", name="kbufc")
        cf0 = load_chunk(ctxt_d, 0, "cf0")
        k_produce(cf0, 0, w["pw_kck"], kvbk, kbuf_c)
        v_c = make_v("v_c")
        v_produce(cf0, 0, w["pw_kcv"], vb_cross, v_c)
        cf1 = load_chunk(ctxt_d, 1, "cf1")
        k_produce(cf1, 1, w["pw_kck"], kvbk, kbuf_c)
        v_produce(cf1, 1, w["pw_kcv"], vb_cross, v_c)

        q1T = ap.tile([128, KD, SH], f32r, tag="qT", bufs=1, name="q1T")

        def consume_q1(m, psum):
            nc.vector.tensor_scalar_add(q1T[:, m, :], psum[:], qb[:, m:m + 1])

        linearT(lambda k: x1T[:, k, :], w["pw_q"], KD, KD, consume_q1)

        a2T = ap.tile([128, KD, SH], f32r, tag="aT", bufs=1, name="a2T")
        attention(q1T, kbuf_c, v_c, a2T, masked=False)

        x2T = resid.tile([128, KD, SH], f32r, tag="resid", name="x2T")
        proj_resid_ln(a2T, w["pw_cp"], cpb, x1T, lng[1], lnb[1], x2T)

    # =================================================== phase 3: MLP
    with tc.tile_pool(name="mlp", bufs=1) as mp:
        h_sb = mp.tile([128, KF, SH], f32r, tag="h", name="h_sb")

        def consume_h(m, psum):
            nc.scalar.activation(h_sb[:, m, :], psum[:], AF.Gelu_apprx_tanh,
                                 bias=fcb[:, m:m + 1], scale=1.0)

        linearT(lambda k: x2T[:, k, :], w["pw_fc"], KF, KD, consume_h)

        r3T = resid.tile([128, KD, SH], f32r, tag="resid", name="r3T")

        def consume_m(m, psum):
            nc.vector.scalar_tensor_tensor(r3T[:, m, :], psum[:],
                                           mpb[:, m:m + 1], x2T[:, m, :],
                                           op0=ALU.add, op1=ALU.add)

        linearT(lambda k: h_sb[:, k, :], w["pw_mp"], KD, KF, consume_m)

        x3T = resid.tile([128, KD, SH], f32, tag="resid", name="x3T")
        layer_norm(r3T, lng[2], lnb[2], x3T)
        nc.sync.dma_start(out_d.rearrange("(k p) t -> p k t", p=128), x3T[:])

    ctx.close()


# ---------------------------------------------------------------- host side
_PROGRAM = None
_PACKED = None


def _get_program():
    global _PROGRAM
    if _PROGRAM is None:
        _PROGRAM = build_program()
    return _PROGRAM


def _build_cmask(parity):
    qb = EV if parity == 0 else OD
    pb = OD if parity == 0 else EV
    kb = qb + pb            # k-tile order of the locally-permuted full x
    m = np.zeros((NKT, 128, 128), np.float32)
    tri = np.zeros((128, 128), np.float32)
    kk, qq = np.meshgrid(np.arange(128), np.arange(128), indexing="ij")
    tri[qq < kk] = MASK_NEG
    for kt in range(NKT):
        gk = kb[kt]
        gq = qb[J0U[kt]]
        if gk == gq:
            m[kt] = tri
        elif gk > gq:
            m[kt] = MASK_NEG
    return m


def _pack_lin(wm, m0, mt, kt):
    """strips in linearT consumption order: mc-chunk-of-4 outer, k inner."""
    out = np.empty(((mt // 4) * kt, 128, 512), np.float32)
    i = 0
    for mc in range(0, mt, 4):
        for k in range(kt):
            out[i] = wm[k * 128:(k + 1) * 128, (m0 + mc) * 128:(m0 + mc + 4) * 128]
            i += 1
    return out


def _pack_v(wm, vcol0):
    """strips in v-loop order: vc outer, k inner."""
    out = np.empty((2 * KD, 128, 512), np.float32)
    i = 0
    for vc in range(2):
        for k in range(KD):
            out[i] = wm[k * 128:(k + 1) * 128, vcol0 + vc * 512:vcol0 + (vc + 1) * 512]
            i += 1
    return out


def _pack_weights(inputs):
    f = lambda n: np.asarray(inputs[n], np.float32)
    packed = {
        "pw_qk": _pack_lin(f("c_attn_w"), 0, 16, KD),
        "pw_vs": _pack_v(f("c_attn_w"), 2 * D),
        "pw_sp": _pack_lin(f("self_proj_w"), 0, KD, KD),
        "pw_q": _pack_lin(f("q_w"), 0, KD, KD),
        "pw_kck": _pack_lin(f("kv_w"), 0, KD, KD),
        "pw_kcv": _pack_v(f("kv_w"), D),
        "pw_cp": _pack_lin(f("cross_proj_w"), 0, KD, KD),
        "pw_fc": _pack_lin(f("fc_w"), 0, KF, KD),
        "pw_mp": _pack_lin(f("mlp_proj_w"), 0, KD, KF),
    }
    for name in BIAS_NAMES:
        packed[name] = np.ascontiguousarray(f(name))
    esel = np.zeros((KD, H, 128), np.float32)
    for m in range(KD):
        esel[m, 2 * m, 0:64] = 1.0
        esel[m, 2 * m + 1, 64:128] = 1.0
    packed["esel"] = esel
    return packed


def kernel(**inputs):
    nc = _get_program()
    x = np.asarray(inputs["x"], np.float32)
    ctx_in = np.asarray(inputs["ctx"], np.float32)
    packed = _pack_weights(inputs)
    in_maps = []
    shard_info = []
    for c in range(N_CORES):
        b, p = c // 2, c % 2
        blocks = (EV + OD) if p == 0 else (OD + EV)
        tok = np.concatenate([np.arange(g * 128, (g + 1) * 128) for g in blocks])
        m = {
            "xt": np.ascontiguousarray(x[b][tok].T),       # [D, S] permuted
            "ctxt": np.ascontiguousarray(ctx_in[b].T),     # [D, S]
            "cmask": _build_cmask(p),
        }
        m.update(packed)
        in_maps.append(m)
        shard_info.append((b, tok[0:SH]))
    res = run_bass_kernel_spmd(nc, in_maps, list(range(N_CORES)))
    out = np.zeros((B, S, D), np.float32)
    for c in range(N_CORES):
        b, tok = shard_info[c]
        out[b, tok, :] = res.results[c]["out"].T
    return out
